# revision 8
# baseline (speedup 1.0000x reference)
"""Trainium2 Bass kernel for nn_InverseResNet (dense MLP with fixed-point blocks).

Computation (per row of x):
  h = x @ W_init + b_init                       # [128] -> [256]
  for b in 4 blocks:  (y = h; x0 = y)
      repeat 10: x <- y - (relu(x @ Wg1[b] + bg1[b]) @ Wg2[b] + bg2[b])
      h = x
  out = h @ W_final + b_final                   # [256] -> [128]

Strategy: pure data parallel over 8 NeuronCores (batch 65536 -> 8192 rows/core).
On-chip layout is feature-major (activations transposed: [feature, batch_col]),
so every matmul contracts over the partition dim with weights stationary:
  out[mf, bcol] += W[kf, mf].T @ actT[kf, bcol]
Batch is processed in 16 tiles of 512 columns; 2 tiles are kept in flight so
PE stays busy while ACT (relu+bias) and DVE (y - g subtract) run.
Matmuls use float32r (1 cycle/row at N=512, ~fp22 precision).
PE-transposes (identity matmul) convert [batch, feat] <-> [feat, batch] at the
input/output boundaries only.
"""

import os
import numpy as np

N_CORES = 8
BATCH, LATENT, HIDDEN, OUT = 65536, 128, 256, 128
NBLOCKS, NITER = 4, 10
B_CORE = BATCH // N_CORES      # 8192
TILE_N = 512                   # batch columns per matmul (1 PSUM bank of fp32)
N_TILES = B_CORE // TILE_N     # 16
PAIR = int(os.environ.get("KERNEL_PAIR", 2))  # batch tiles in flight

_CACHE = {}


def _build(n_tiles=N_TILES):
    from contextlib import ExitStack
    import concourse.bacc as bacc
    import concourse.tile as tile
    import concourse.mybir as mybir
    from concourse.masks import make_identity

    f32 = mybir.dt.float32
    f32r = mybir.dt.float32r
    AF = mybir.ActivationFunctionType

    nc = bacc.Bacc("TRN2", target_bir_lowering=False, debug=False,
                   num_devices=N_CORES)

    x_d = nc.dram_tensor("x", [B_CORE, LATENT], f32, kind="ExternalInput").ap()
    w1_d = nc.dram_tensor("w1", [128, NBLOCKS, 2, HIDDEN], f32, kind="ExternalInput").ap()
    w2_d = nc.dram_tensor("w2", [128, NBLOCKS, 2, HIDDEN], f32, kind="ExternalInput").ap()
    b1_d = nc.dram_tensor("b1", [128, NBLOCKS, 2], f32, kind="ExternalInput").ap()
    b2_d = nc.dram_tensor("b2", [128, NBLOCKS, 2], f32, kind="ExternalInput").ap()
    wi_d = nc.dram_tensor("wi", [128, HIDDEN], f32, kind="ExternalInput").ap()
    bi_d = nc.dram_tensor("bi", [128, 2], f32, kind="ExternalInput").ap()
    wf_d = nc.dram_tensor("wf", [128, 2, OUT], f32, kind="ExternalInput").ap()
    bf_d = nc.dram_tensor("bf", [128, 1], f32, kind="ExternalInput").ap()
    y_d = nc.dram_tensor("y", [B_CORE, OUT], f32, kind="ExternalOutput").ap()

    def r(ap):
        return ap.bitcast(f32r)

    with tile.TileContext(nc) as tc, ExitStack() as ctx:
        wp = ctx.enter_context(tc.tile_pool(name="weights", bufs=1))
        ap_ = ctx.enter_context(tc.tile_pool(name="acts", bufs=PAIR))
        tp_ = ctx.enter_context(tc.tile_pool(name="tbuf", bufs=2 * PAIR))
        iop = ctx.enter_context(tc.tile_pool(name="io", bufs=PAIR + 1))
        pp = ctx.enter_context(tc.tile_pool(name="psum", bufs=8, space="PSUM"))

        ident = wp.tile([128, 128], f32)
        make_identity(nc, ident)
        wi_s = wp.tile([128, HIDDEN], f32r)
        nc.sync.dma_start(out=wi_s, in_=r(wi_d))
        bi_s = wp.tile([128, 2], f32)
        nc.sync.dma_start(out=bi_s, in_=bi_d)
        w1_s = wp.tile([128, NBLOCKS, 2, HIDDEN], f32r)
        nc.sync.dma_start(out=w1_s, in_=r(w1_d))
        w2_s = wp.tile([128, NBLOCKS, 2, HIDDEN], f32r)
        nc.sync.dma_start(out=w2_s, in_=r(w2_d))
        b1_s = wp.tile([128, NBLOCKS, 2], f32)
        nc.sync.dma_start(out=b1_s, in_=b1_d)
        b2_s = wp.tile([128, NBLOCKS, 2], f32)
        nc.sync.dma_start(out=b2_s, in_=b2_d)
        wf_s = wp.tile([128, 2, OUT], f32r)
        nc.sync.dma_start(out=wf_s, in_=r(wf_d))
        bf_s = wp.tile([128, 1], f32)
        nc.sync.dma_start(out=bf_s, in_=bf_d)

        x_view = x_d.rearrange("(t j p) f -> t p j f", p=128, j=4)
        y_view = y_d.rearrange("(t j p) f -> t p j f", p=128, j=4)

        def stage_in(t):
            """Load tile t, transpose to feature-major, apply init layer.
            Returns (h0, h1): the two 128-feature chunks of h, [128, TILE_N]."""
            xin = iop.tile([128, 4, 128], f32, tag="xin")
            nc.sync.dma_start(out=xin, in_=x_view[t])
            pt = pp.tile([128, 4, 128], f32, tag="ps")
            for j in range(4):
                nc.tensor.matmul(pt[:, j, :], xin[:, j, :], ident,
                                 start=(j == 0), stop=(j == 3),
                                 is_transpose=True)
            xt = iop.tile([128, 4, 128], f32r, tag="xt")
            nc.vector.tensor_copy(out=xt, in_=pt)
            xtf = xt.rearrange("p j f -> p (j f)")
            hs = []
            for m in range(2):
                ps = pp.tile([128, TILE_N], f32, tag="ps")
                nc.tensor.matmul(ps, wi_s[:, m * 128:(m + 1) * 128], xtf,
                                 start=True, stop=True)
                h = ap_.tile([128, TILE_N], f32r, tag=f"h{m}")
                nc.scalar.activation(out=h, in_=ps, func=AF.Identity,
                                     bias=bi_s[:, m:m + 1], scale=1.0)
                hs.append(h)
            return hs

        def make_yprime(st, blk):
            """yp = h - bg2[blk]  (fold the second bias into the y constant)."""
            yps = []
            for m in range(2):
                yp = ap_.tile([128, TILE_N], f32, tag=f"yp{m}")
                nc.vector.tensor_scalar_sub(yp, st[m], b2_s[:, blk, m:m + 1])
                yps.append(yp)
            return yps

        def emit_half1(hs, blk):
            """mm1 + relu for one tile; returns t chunks."""
            ts = []
            for m in range(2):
                ps = pp.tile([128, TILE_N], f32, tag="ps")
                nc.tensor.matmul(ps, w1_s[:, blk, 0, m * 128:(m + 1) * 128],
                                 hs[0], start=True, stop=False)
                nc.tensor.matmul(ps, w1_s[:, blk, 1, m * 128:(m + 1) * 128],
                                 hs[1], start=False, stop=True)
                t = tp_.tile([128, TILE_N], f32r, tag=f"t{m}", name=f"t{m}")
                nc.scalar.activation(out=t, in_=ps, func=AF.Relu,
                                     bias=b1_s[:, blk, m:m + 1], scale=1.0)
                ts.append(t)
            return ts

        def emit_half2(hs, yps, ts, blk):
            """mm2 + subtract for one tile; overwrites hs in place."""
            for m in range(2):
                ps = pp.tile([128, TILE_N], f32, tag="ps")
                nc.tensor.matmul(ps, w2_s[:, blk, 0, m * 128:(m + 1) * 128],
                                 ts[0], start=True, stop=False)
                nc.tensor.matmul(ps, w2_s[:, blk, 1, m * 128:(m + 1) * 128],
                                 ts[1], start=False, stop=True)
                nc.vector.tensor_sub(out=hs[m], in0=yps[m], in1=ps)

        def stage_out(t, hs):
            ps = pp.tile([128, TILE_N], f32, tag="ps")
            nc.tensor.matmul(ps, wf_s[:, 0, :], hs[0], start=True, stop=False)
            nc.tensor.matmul(ps, wf_s[:, 1, :], hs[1], start=False, stop=True)
            ot = iop.tile([128, TILE_N], f32, tag="ot")
            nc.scalar.activation(out=ot, in_=ps, func=AF.Identity,
                                 bias=bf_s[:, 0:1], scale=1.0)
            po = pp.tile([128, 4, 128], f32, tag="ps")
            for j in range(4):
                nc.tensor.matmul(po[:, j, :], ot[:, j * 128:(j + 1) * 128], ident,
                                 start=(j == 0), stop=(j == 3),
                                 is_transpose=True)
            on = iop.tile([128, 4, 128], f32, tag="on")
            nc.vector.tensor_copy(out=on, in_=po)
            nc.sync.dma_start(out=y_view[t], in_=on)

        for pair in range(0, n_tiles, PAIR):
            tiles = list(range(pair, min(pair + PAIR, n_tiles)))
            states = [stage_in(t) for t in tiles]
            for blk in range(NBLOCKS):
                yps = [make_yprime(st, blk) for st in states]
                for _ in range(NITER):
                    tss = [emit_half1(st, blk) for st in states]
                    for st, yp, ts2 in zip(states, yps, tss):
                        emit_half2(st, yp, ts2, blk)
            for t, st in zip(tiles, states):
                stage_out(t, st)

    nc.compile()
    return nc


def _prep_weights(W_init, b_init, Wg1, bg1, Wg2, bg2, W_final, b_final):
    f = np.float32
    return {
        "wi": np.ascontiguousarray(np.asarray(W_init, f)),
        "bi": np.ascontiguousarray(np.asarray(b_init, f).reshape(2, 128).T),
        "w1": np.ascontiguousarray(
            np.asarray(Wg1, f).reshape(NBLOCKS, 2, 128, HIDDEN).transpose(2, 0, 1, 3)),
        "w2": np.ascontiguousarray(
            np.asarray(Wg2, f).reshape(NBLOCKS, 2, 128, HIDDEN).transpose(2, 0, 1, 3)),
        "b1": np.ascontiguousarray(
            np.asarray(bg1, f).reshape(NBLOCKS, 2, 128).transpose(2, 0, 1)),
        "b2": np.ascontiguousarray(
            np.asarray(bg2, f).reshape(NBLOCKS, 2, 128).transpose(2, 0, 1)),
        "wf": np.ascontiguousarray(
            np.asarray(W_final, f).reshape(2, 128, OUT).transpose(1, 0, 2)),
        "bf": np.ascontiguousarray(np.asarray(b_final, f).reshape(128, 1)),
    }


def kernel(x, W_init, b_init, Wg1, bg1, Wg2, bg2, W_final, b_final):
    from concourse.bass_utils import run_bass_kernel_spmd

    n_tiles = int(os.environ.get("KERNEL_N_TILES", N_TILES))
    key = ("nc", n_tiles)
    if key not in _CACHE:
        _CACHE[key] = _build(n_tiles)
    nc = _CACHE[key]

    w = _prep_weights(W_init, b_init, Wg1, bg1, Wg2, bg2, W_final, b_final)
    x = np.ascontiguousarray(np.asarray(x, np.float32))
    shards = x.reshape(N_CORES, B_CORE, LATENT)
    in_maps = [dict(w, x=np.ascontiguousarray(shards[i])) for i in range(N_CORES)]

    res = run_bass_kernel_spmd(nc, in_maps, core_ids=list(range(N_CORES)))
    y = np.concatenate([res.results[i]["y"] for i in range(N_CORES)], axis=0)
    return y.astype(np.float32)


# revision 10
# speedup vs baseline: 1.3544x; 1.3544x over previous
"""Trainium2 Bass kernel for nn_InverseResNet (dense MLP with fixed-point blocks).

Computation (per row of x):
  h = x @ W_init + b_init                       # [128] -> [256]
  for b in 4 blocks:  (y = h; x0 = y)
      repeat 10: x <- y - (relu(x @ Wg1[b] + bg1[b]) @ Wg2[b] + bg2[b])
      h = x
  out = h @ W_final + b_final                   # [256] -> [128]

Strategy: pure data parallel over 8 NeuronCores (batch 65536 -> 8192 rows/core).
On-chip layout is feature-major (activations transposed: [feature, batch_col]),
so every matmul contracts over the partition dim with weights stationary:
  out[mf, bcol] += W[kf, mf].T @ actT[kf, bcol]
Batch is processed in 16 tiles of 512 columns; 2 tiles are kept in flight so
PE stays busy while ACT (relu+bias) and DVE (y - g subtract) run.
Matmuls use float32r (1 cycle/row at N=512, ~fp22 precision).
PE-transposes (identity matmul) convert [batch, feat] <-> [feat, batch] at the
input/output boundaries only.
"""

import os
import numpy as np

N_CORES = 8
BATCH, LATENT, HIDDEN, OUT = 65536, 128, 256, 128
NBLOCKS, NITER = 4, 10
B_CORE = BATCH // N_CORES      # 8192
TILE_N = 512                   # batch columns per matmul (1 PSUM bank of fp32)
N_TILES = B_CORE // TILE_N     # 16
PAIR = int(os.environ.get("KERNEL_PAIR", 2))  # batch tiles in flight

_CACHE = {}


def _build(n_tiles=N_TILES):
    from contextlib import ExitStack
    import concourse.bacc as bacc
    import concourse.tile as tile
    import concourse.mybir as mybir
    from concourse.masks import make_identity

    f32 = mybir.dt.float32
    f32r = mybir.dt.float32r
    AF = mybir.ActivationFunctionType

    nc = bacc.Bacc("TRN2", target_bir_lowering=False, debug=False,
                   num_devices=N_CORES)

    x_d = nc.dram_tensor("x", [B_CORE, LATENT], f32, kind="ExternalInput").ap()
    w1_d = nc.dram_tensor("w1", [128, NBLOCKS, 2, HIDDEN], f32, kind="ExternalInput").ap()
    w2_d = nc.dram_tensor("w2", [128, NBLOCKS, 2, HIDDEN], f32, kind="ExternalInput").ap()
    mn_d = nc.dram_tensor("mn", [128, NBLOCKS, 2, HIDDEN], f32, kind="ExternalInput").ap()
    e_d = nc.dram_tensor("e", [128, NBLOCKS, 2], f32, kind="ExternalInput").ap()
    b1_d = nc.dram_tensor("b1", [128, NBLOCKS, 2], f32, kind="ExternalInput").ap()
    b2_d = nc.dram_tensor("b2", [128, NBLOCKS, 2], f32, kind="ExternalInput").ap()
    wi_d = nc.dram_tensor("wi", [128, HIDDEN], f32, kind="ExternalInput").ap()
    bi_d = nc.dram_tensor("bi", [128, 2], f32, kind="ExternalInput").ap()
    wf_d = nc.dram_tensor("wf", [128, 2, OUT], f32, kind="ExternalInput").ap()
    bf_d = nc.dram_tensor("bf", [128, 1], f32, kind="ExternalInput").ap()
    y_d = nc.dram_tensor("y", [B_CORE, OUT], f32, kind="ExternalOutput").ap()

    def r(ap):
        return ap.bitcast(f32r)

    with tile.TileContext(nc) as tc, ExitStack() as ctx:
        wp = ctx.enter_context(tc.tile_pool(name="weights", bufs=1))
        ap_ = ctx.enter_context(tc.tile_pool(name="acts", bufs=2 * PAIR))
        tp_ = ctx.enter_context(tc.tile_pool(name="tbuf", bufs=2 * PAIR))
        iop = ctx.enter_context(tc.tile_pool(name="io", bufs=PAIR + 1))
        pp = ctx.enter_context(tc.tile_pool(name="psum", bufs=8, space="PSUM"))

        ident = wp.tile([128, 128], f32)
        make_identity(nc, ident)
        wi_s = wp.tile([128, HIDDEN], f32r)
        nc.sync.dma_start(out=wi_s, in_=r(wi_d))
        bi_s = wp.tile([128, 2], f32)
        nc.sync.dma_start(out=bi_s, in_=bi_d)
        w1_s = wp.tile([128, NBLOCKS, 2, HIDDEN], f32r)
        nc.sync.dma_start(out=w1_s, in_=r(w1_d))
        w2_s = wp.tile([128, NBLOCKS, 2, HIDDEN], f32r)
        nc.sync.dma_start(out=w2_s, in_=r(w2_d))
        b1_s = wp.tile([128, NBLOCKS, 2], f32)
        nc.sync.dma_start(out=b1_s, in_=b1_d)
        b2_s = wp.tile([128, NBLOCKS, 2], f32)
        nc.sync.dma_start(out=b2_s, in_=b2_d)
        mn_s = wp.tile([128, NBLOCKS, 2, HIDDEN], f32r)
        nc.sync.dma_start(out=mn_s, in_=r(mn_d))
        e_s = wp.tile([128, NBLOCKS, 2], f32)
        nc.sync.dma_start(out=e_s, in_=e_d)
        identr = wp.tile([128, 128], f32r)
        nc.vector.tensor_copy(out=identr, in_=ident)
        wf_s = wp.tile([128, 2, OUT], f32r)
        nc.sync.dma_start(out=wf_s, in_=r(wf_d))
        bf_s = wp.tile([128, 1], f32)
        nc.sync.dma_start(out=bf_s, in_=bf_d)

        x_view = x_d.rearrange("(t j p) f -> t p j f", p=128, j=4)
        y_view = y_d.rearrange("(t j p) f -> t p j f", p=128, j=4)

        def stage_in(t):
            """Load tile t, transpose to feature-major, apply init layer.
            Returns (h0, h1): the two 128-feature chunks of h, [128, TILE_N]."""
            xin = iop.tile([128, 4, 128], f32, tag="xin")
            nc.sync.dma_start(out=xin, in_=x_view[t])
            pt = pp.tile([128, 4, 128], f32, tag="ps")
            for j in range(4):
                nc.tensor.matmul(pt[:, j, :], xin[:, j, :], ident,
                                 start=(j == 0), stop=(j == 3),
                                 is_transpose=True)
            xt = iop.tile([128, 4, 128], f32r, tag="xt")
            nc.vector.tensor_copy(out=xt, in_=pt)
            xtf = xt.rearrange("p j f -> p (j f)")
            hs = []
            for m in range(2):
                ps = pp.tile([128, TILE_N], f32, tag="ps")
                nc.tensor.matmul(ps, wi_s[:, m * 128:(m + 1) * 128], xtf,
                                 start=True, stop=True)
                h = ap_.tile([128, TILE_N], f32r, tag=f"h{m}")
                nc.scalar.activation(out=h, in_=ps, func=AF.Identity,
                                     bias=bi_s[:, m:m + 1], scale=1.0)
                hs.append(h)
            return hs

        from concourse.alu_op_type import AluOpType

        def emit_block_head(st, blk):
            """P = W1^T h; c = P + e (iteration constant); t0 = relu(P + b1)."""
            cs, t0s = [], []
            for m in range(2):
                ps = pp.tile([128, TILE_N], f32, tag="ps")
                nc.tensor.matmul(ps, w1_s[:, blk, 0, m * 128:(m + 1) * 128],
                                 st[0], start=True, stop=False)
                nc.tensor.matmul(ps, w1_s[:, blk, 1, m * 128:(m + 1) * 128],
                                 st[1], start=False, stop=True)
                c = ap_.tile([128, TILE_N], f32r, tag=f"c{m}", name=f"c{m}")
                nc.scalar.activation(out=c, in_=ps, func=AF.Identity,
                                     bias=e_s[:, blk, m:m + 1], scale=1.0)
                t0 = tp_.tile([128, TILE_N], f32r, tag=f"t{m}", name=f"t{m}")
                nc.scalar.activation(out=t0, in_=ps, func=AF.Relu,
                                     bias=b1_s[:, blk, m:m + 1], scale=1.0)
                cs.append(c)
                t0s.append(t0)
            return cs, t0s

        def emit_iter(cs, ts, blk, par):
            """t <- relu(c - M^T t) in one PSUM group: identity-matmul adds c,
            mn (= -W2@W1) accumulates on top; relu alternates ACT/DVE."""
            nts = []
            for m in range(2):
                ps = pp.tile([128, TILE_N], f32, tag="ps")
                nc.tensor.matmul(ps, identr, cs[m], start=True, stop=False)
                nc.tensor.matmul(ps, mn_s[:, blk, 0, m * 128:(m + 1) * 128],
                                 ts[0], start=False, stop=False)
                nc.tensor.matmul(ps, mn_s[:, blk, 1, m * 128:(m + 1) * 128],
                                 ts[1], start=False, stop=True)
                t = tp_.tile([128, TILE_N], f32r, tag=f"t{m}", name=f"t{m}")
                if (par + m) % 2 == 0:
                    nc.scalar.activation(out=t, in_=ps, func=AF.Relu,
                                         bias=0.0, scale=1.0)
                else:
                    nc.vector.tensor_scalar_max(t, ps, 0.0)
                nts.append(t)
            return nts

        def emit_block_tail(st, ts, blk):
            """h <- (W2n^T t9 - b2) + h   (W2n = -W2)."""
            for m in range(2):
                ps = pp.tile([128, TILE_N], f32, tag="ps")
                nc.tensor.matmul(ps, w2_s[:, blk, 0, m * 128:(m + 1) * 128],
                                 ts[0], start=True, stop=False)
                nc.tensor.matmul(ps, w2_s[:, blk, 1, m * 128:(m + 1) * 128],
                                 ts[1], start=False, stop=True)
                h = ap_.tile([128, TILE_N], f32r, tag=f"h{m}", name=f"h{m}")
                nc.vector.scalar_tensor_tensor(
                    out=h, in0=ps, scalar=b2_s[:, blk, m:m + 1], in1=st[m],
                    op0=AluOpType.subtract, op1=AluOpType.add)
                st[m] = h

        def stage_out(t, hs):
            ps = pp.tile([128, TILE_N], f32, tag="ps")
            nc.tensor.matmul(ps, wf_s[:, 0, :], hs[0], start=True, stop=False)
            nc.tensor.matmul(ps, wf_s[:, 1, :], hs[1], start=False, stop=True)
            ot = iop.tile([128, TILE_N], f32, tag="ot")
            nc.scalar.activation(out=ot, in_=ps, func=AF.Identity,
                                 bias=bf_s[:, 0:1], scale=1.0)
            po = pp.tile([128, 4, 128], f32, tag="ps")
            for j in range(4):
                nc.tensor.matmul(po[:, j, :], ot[:, j * 128:(j + 1) * 128], ident,
                                 start=(j == 0), stop=(j == 3),
                                 is_transpose=True)
            on = iop.tile([128, 4, 128], f32, tag="on")
            nc.vector.tensor_copy(out=on, in_=po)
            nc.sync.dma_start(out=y_view[t], in_=on)

        for pair in range(0, n_tiles, PAIR):
            tiles = list(range(pair, min(pair + PAIR, n_tiles)))
            states = [stage_in(t) for t in tiles]
            for blk in range(NBLOCKS):
                heads = [emit_block_head(st, blk) for st in states]
                cs_all = [h[0] for h in heads]
                ts_all = [h[1] for h in heads]
                for k in range(1, NITER):
                    ts_all = [emit_iter(cs, ts, blk, k + i)
                              for i, (cs, ts) in enumerate(zip(cs_all, ts_all))]
                for st, ts in zip(states, ts_all):
                    emit_block_tail(st, ts, blk)
            for t, st in zip(tiles, states):
                stage_out(t, st)

    nc.compile()
    return nc


def _prep_weights(W_init, b_init, Wg1, bg1, Wg2, bg2, W_final, b_final):
    f = np.float32
    w1_64 = np.asarray(Wg1, np.float64)
    w2_64 = np.asarray(Wg2, np.float64)
    mn = -np.einsum("bij,bjk->bik", w2_64, w1_64)          # -(W2 @ W1) per block
    e = np.asarray(bg1, np.float64) - np.einsum("bj,bjk->bk", np.asarray(bg2, np.float64), w1_64)
    return {
        "wi": np.ascontiguousarray(np.asarray(W_init, f)),
        "bi": np.ascontiguousarray(np.asarray(b_init, f).reshape(2, 128).T),
        "w1": np.ascontiguousarray(
            np.asarray(Wg1, f).reshape(NBLOCKS, 2, 128, HIDDEN).transpose(2, 0, 1, 3)),
        "w2": np.ascontiguousarray(
            (-np.asarray(Wg2, f)).reshape(NBLOCKS, 2, 128, HIDDEN).transpose(2, 0, 1, 3)),
        "mn": np.ascontiguousarray(
            np.asarray(mn, f).reshape(NBLOCKS, 2, 128, HIDDEN).transpose(2, 0, 1, 3)),
        "e": np.ascontiguousarray(
            np.asarray(e, f).reshape(NBLOCKS, 2, 128).transpose(2, 0, 1)),
        "b1": np.ascontiguousarray(
            np.asarray(bg1, f).reshape(NBLOCKS, 2, 128).transpose(2, 0, 1)),
        "b2": np.ascontiguousarray(
            np.asarray(bg2, f).reshape(NBLOCKS, 2, 128).transpose(2, 0, 1)),
        "wf": np.ascontiguousarray(
            np.asarray(W_final, f).reshape(2, 128, OUT).transpose(1, 0, 2)),
        "bf": np.ascontiguousarray(np.asarray(b_final, f).reshape(128, 1)),
    }


def kernel(x, W_init, b_init, Wg1, bg1, Wg2, bg2, W_final, b_final):
    from concourse.bass_utils import run_bass_kernel_spmd

    n_tiles = int(os.environ.get("KERNEL_N_TILES", N_TILES))
    key = ("nc", n_tiles)
    if key not in _CACHE:
        _CACHE[key] = _build(n_tiles)
    nc = _CACHE[key]

    w = _prep_weights(W_init, b_init, Wg1, bg1, Wg2, bg2, W_final, b_final)
    x = np.ascontiguousarray(np.asarray(x, np.float32))
    shards = x.reshape(N_CORES, B_CORE, LATENT)
    in_maps = [dict(w, x=np.ascontiguousarray(shards[i])) for i in range(N_CORES)]

    res = run_bass_kernel_spmd(nc, in_maps, core_ids=list(range(N_CORES)))
    y = np.concatenate([res.results[i]["y"] for i in range(N_CORES)], axis=0)
    return y.astype(np.float32)
